# revision 91
# baseline (speedup 1.0000x reference)
"""Trainium2 Bass kernel for nn_LocalMQA (S=2048, D_MODEL=1024, H=16, D=64, WIN=128).

Sharding: sequence-parallel across 8 cores (256 output rows each) with a
128-row halo recomputed for k/v. No collectives; each core produces a
disjoint slice of the output.

Per-core pipeline (all fp16 matmuls, PSUM accumulates f32):
  - inputs DMA'd as large fully-contiguous per-partition transfers, split
    across the SP and ACT HWDGE rings; w2 queued behind w1 (needed last).
  - qkv: outer loop over K-chunks so PE consumes weight chunks as they
    stream in; all 9 PSUM accumulators open simultaneously.
  - attention per (it, h): maskbias copied into PSUM by DVE/ACT (not PE),
    score matmul accumulates on top; DVE rowmax(negate); ACT Exp with
    accum_out giving the softmax denominator Z for free; PE transpose;
    evac copy; AV matmul; batched DVE reciprocal of Z; scale-evac to o16.
  - per-it: o16 transposed to oT, outproj for that it runs immediately
    (overlaps the other it's attention), out DMA'd per (nt, it).
Host transposes/concats/casts the 8 outT slices into the final (2048, 1024).
"""
import contextlib

import numpy as np

import concourse.bacc as bacc
import concourse.mybir as mybir
import concourse.tile as tile
from concourse.bass_utils import run_bass_kernel_spmd

S = 2048
DM = 1024
H = 16
D = 64
WIN = 128
NC = 8
RPC = S // NC          # rows per core = 256
HALO = 128
XW = RPC + HALO        # per-core xT width = 384

F32 = mybir.dt.float32
F16 = mybir.dt.float16

_CACHED = {}


def _build(debug=False):
    nc = bacc.Bacc("TRN2", target_bir_lowering=False, debug=False, num_devices=NC)

    xT_d = nc.dram_tensor("xT", [128, 8 * XW], F16, kind="ExternalInput").ap()
    # w1 tile-major: [p, out-tile(9: kv,q0..q7), K-chunk(8), 128]
    w1_d = nc.dram_tensor("w1T", [128, 9 * 8 * 128], F16, kind="ExternalInput").ap()
    w2_d = nc.dram_tensor("w2T", [128, 8 * 1024], F16, kind="ExternalInput").ap()
    b1_d = nc.dram_tensor("b1", [128, 9], F32, kind="ExternalInput").ap()
    b2_d = nc.dram_tensor("b2", [128, 8], F32, kind="ExternalInput").ap()
    msk_d = nc.dram_tensor("mask", [128, 2, 512], F16, kind="ExternalInput").ap()
    id16_d = nc.dram_tensor("ident16", [128, 128], F16, kind="ExternalInput").ap()
    out_d = nc.dram_tensor("outT", [8, 2, 128, 128], F16, kind="ExternalOutput").ap()

    AF = mybir.ActivationFunctionType
    if debug:
        dbg = {
            "dbg_kv": nc.dram_tensor("dbg_kv", [128, XW], F16, kind="ExternalOutput").ap(),
            "dbg_q": nc.dram_tensor("dbg_q", [128, 8, 256], F16, kind="ExternalOutput").ap(),
            "dbg_v16t": nc.dram_tensor("dbg_v16t", [128, 3, 64], F16, kind="ExternalOutput").ap(),
            "dbg_attn": nc.dram_tensor("dbg_attn", [128, 2, 256], F16, kind="ExternalOutput").ap(),
            "dbg_o16": nc.dram_tensor("dbg_o16", [128, 16, 64], F16, kind="ExternalOutput").ap(),
        }

    with tile.TileContext(nc) as tc:
      with (
        tc.tile_pool(name="w", bufs=1) as wp,      # weights + constants
        tc.tile_pool(name="act", bufs=1) as ap_,   # persistent activations
        tc.tile_pool(name="sm", bufs=8) as smp,    # small softmax tiles
        tc.tile_pool(name="att", bufs=6) as attp,
        tc.tile_pool(name="attn_b", bufs=18) as attnp,  # buffered exp tiles
        tc.tile_pool(name="o16p", bufs=1) as o16p,
        tc.tile_pool(name="outp", bufs=4) as outp,
      ):
        xT = wp.tile([128, 8, XW], F16)
        w1 = wp.tile([128, 9, 8, 128], F16)
        w2 = wp.tile([128, 8, 1024], F16)
        b1 = wp.tile([128, 9], F32)
        b2 = wp.tile([128, 8], F32)
        msk = wp.tile([128, 2, 512], F16)
        id16 = wp.tile([128, 128], F16)

        # DMA plan: small constants are static-preloaded by the runtime
        # (arrive ~2.6us); the two HWDGE rings carry the big tensors in
        # consumption order (x, kv weights, mask, q tiles), w2 last.
        xTf = xT[:].rearrange("p c n -> p (c n)")
        w1f = w1[:].rearrange("p t c n -> p (t c n)")
        w2f = w2[:].rearrange("p c n -> p (c n)")
        nc.sync.dma_start(id16[:], id16_d)   # first: unblocks the PE warm-up
        nc.gpsimd.dma_start(b1[:], b1_d)
        nc.gpsimd.dma_start(b2[:], b2_d)
        nc.sync.dma_start(xTf[:, 0:4 * XW], xT_d[:, 0:4 * XW])
        nc.scalar.dma_start(xTf[:, 4 * XW:8 * XW], xT_d[:, 4 * XW:8 * XW])
        nc.sync.dma_start(w1f[:, 0:1024], w1_d[:, 0:1024])          # kv tile
        nc.scalar.dma_start(w1f[:, 1024:2048], w1_d[:, 1024:2048])  # q0
        nc.sync.dma_start(msk[:], msk_d)
        for t in range(1, 8):  # q1..q7 alternate across the HWDGE rings
            eng = nc.scalar if t % 2 == 0 else nc.sync
            eng.dma_start(w1f[:, (t + 1) * 1024:(t + 2) * 1024],
                          w1_d[:, (t + 1) * 1024:(t + 2) * 1024])
        # (w2 is queued on the rings after the v16t transposes below)

        # PE warm-up: the tensor engine p-state ramps to full clock after
        # ~3us of continuous execution. An iota-generated tile is available
        # ~6us before the first DMA lands, so the ramp completes before the
        # first real matmul.
        wsrc = wp.tile([128, 128], F16)
        nc.vector.memset(wsrc[:], 1.0)
        with tc.tile_pool(name="warm", bufs=1, space="PSUM") as wmp:
            wtile = wmp.tile([128, 128], F16)
            for _ in range(16):
                nc.tensor.transpose(wtile[:], wsrc[:], wsrc[:])

        # ---- qkv projection: consume w1 K-chunks as they arrive ----
        kv_sb = ap_.tile([128, XW], F16)       # k rows 0:64 (+ mirror 64:128)
        v16r = ap_.tile([128, 3, 128], F16)    # v (biased) at partitions 64:128
        v16s = ap_.tile([128, 3, 64], F16)     # v^T staging (DMA transpose dst)
        v16t = ap_.tile([128, 3, 65], F16)     # v^T blocks [key, d] + ones col
        q_sb = ap_.tile([128, 8, 256], F16)    # q tiles, 2 heads per tile
        nc.vector.memset(v16t[:, :, 64:65], 1.0)

        with contextlib.nullcontext():
            # The attention pipeline: stage A (scores+rowmax+exp), stage B
            # (transpose+evac), stage C (AV+normalize+streamed outproj).
            # Pair hp of tile it is exactly outproj rhs chunk hp, so the
            # output projection accumulates per-pair into two packed PSUM
            # banks and the kernel has no outproj tail.
            rts = [o16p.tile([128, 16], F32, tag=f"r_{i}", name=f"rt_{i}")
                   for i in range(2)]
            if debug:
                dbg_o16 = o16p.tile([128, 16, 64], F16, tag="dbg_o16",
                                    name="dbg_o16t")
            st = {}

            def stage_a(hh):
                it, h = divmod(hh, 16)
                if h % 2 == 0:
                    scb = ps_s.tile([128, 512], F32, tag="sc", name="scb")
                    # one inject fills mask for BOTH packed sc slots
                    # (start=True zeroes the whole bank, so it must be the
                    # bank's first write each rotation)
                    nc.tensor.matmul(scb[:], id16[:], msk[:, it, :],
                                     start=True, stop=False)
                    st["scb"] = scb
                sc = st["scb"][:, (h % 2) * 256:(h % 2) * 256 + 256]
                nc.tensor.matmul(
                    sc[:],
                    q_sb[64 * (h % 2):64 * (h % 2) + 64, h // 2,
                         it * 128:it * 128 + 128],
                    kv_sb[64 * (h % 2):64 * (h % 2) + 64,
                          it * 128:it * 128 + 256],
                    start=False, stop=(h % 2 == 1), skip_group_check=True)
                negm = smp.tile([128, 1], F32, tag="negm")
                nc.vector.tensor_reduce(negm[:], sc[:],
                                        axis=mybir.AxisListType.X,
                                        op=mybir.AluOpType.max, negate=True)
                attn = attp.tile([128, 256], F16, tag="attn")
                nc.scalar.activation(attn[:], sc[:], AF.Exp,
                                     bias=negm[:], scale=1.0)
                if debug and it == 0 and h < 2:
                    nc.gpsimd.dma_start(dbg["dbg_attn"][:, h, :], attn[:])
                st[("attn", hh)] = attn

            # ---- phase 1: qkv projection + it0 stage A ----
            with (
                tc.tile_pool(name="ps_q", bufs=3, space="PSUM") as ps_q,
                tc.tile_pool(name="ps_kv", bufs=1, space="PSUM") as ps_kv,
            ):
                kvp = ps_kv.tile([128, XW], F32)
                for c in range(8):
                    nc.tensor.matmul(kvp[:], w1[:, 0, c, :], xT[:, c, :],
                                     start=(c == 0), stop=(c == 7))
                nc.scalar.activation(kv_sb[0:64, :], kvp[0:64, :],
                                     AF.Identity, bias=b1[0:64, 0:1],
                                     scale=1.0)
                nc.vector.tensor_scalar_add(
                    v16r[64:128, :, :].rearrange("p b n -> p (b n)"),
                    kvp[64:128, :], b1[64:128, 0:1])
                for t in range(8):
                    qp = ps_q.tile([128, 256], F32, tag="qp")
                    for c in range(8):
                        nc.tensor.matmul(qp[:], w1[:, t + 1, c, :],
                                         xT[:, c, HALO:XW],
                                         start=(c == 0), stop=(c == 7))
                    if t % 2 == 0:
                        nc.vector.tensor_scalar_add(q_sb[:, t, :], qp[:],
                                                    b1[:, t + 1:t + 2])
                    else:
                        nc.scalar.activation(q_sb[:, t, :], qp[:],
                                             AF.Identity,
                                             bias=b1[:, t + 1:t + 2],
                                             scale=1.0)


            # v^T blocks via DMA transpose (v16r is ready mid-phase-1)
            for b in range(3):
                nc.sync.dma_start(v16s[:, b, :], v16r[64:128, b, :],
                                  transpose=True)
            nc.vector.tensor_copy(v16t[:, :, 0:64], v16s[:])
            nc.gpsimd.dma_start(kv_sb[64:128, :], kv_sb[0:64, :])
            # w2 rides behind the v16t transposes; needed only by outproj
            nc.sync.dma_start(w2f[:, 0:4096], w2_d[:, 0:4096])
            nc.scalar.dma_start(w2f[:, 4096:8192], w2_d[:, 4096:8192])
            if debug:
                nc.gpsimd.dma_start(dbg["dbg_kv"], kv_sb[:])
                nc.gpsimd.dma_start(dbg["dbg_v16t"], v16s[:])
                nc.gpsimd.dma_start(dbg["dbg_q"], q_sb[:])

            _stk = contextlib.ExitStack()
            ps_s = _stk.enter_context(
                tc.tile_pool(name="ps_s", bufs=2, space="PSUM"))
            ps_o = _stk.enter_context(
                tc.tile_pool(name="ps_o", bufs=2, space="PSUM"))
            ps_t = _stk.enter_context(
                tc.tile_pool(name="ps_t", bufs=2, space="PSUM"))
            ps_f = _stk.enter_context(
                tc.tile_pool(name="ps_f", bufs=1, space="PSUM"))

            def stage_b(pp):
                # one transpose+evac round for a PAIR of heads (2h, 2h+1)
                it, hp = divmod(pp, 8)
                ptt = ps_t.tile([128, 4, 128], F16, tag="tp")
                for j in range(2):
                    attn = st.pop(("attn", pp * 2 + j))
                    for b in range(2):
                        nc.tensor.transpose(ptt[:, 2 * j + b, :],
                                            attn[:, b * 128:b * 128 + 128],
                                            id16[:])
                attnT = attp.tile([128, 4, 128], F16, tag="attnT")
                nc.vector.tensor_copy(
                    attnT[:].rearrange("p a b -> p (a b)"),
                    ptt[:].rearrange("p a b -> p (a b)"))
                st[("attnT", pp)] = attnT

            def stage_c(pp):
                it, hp = divmod(pp, 8)
                attnT = st.pop(("attnT", pp))
                # both heads' AV share one PSUM bank: head A's start=True
                # zeroes the whole bank, head B accumulates from zero
                po = ps_o.tile([128, 2, 65], F32, tag="po")
                for j in range(2):
                    for b in range(2):
                        nc.tensor.matmul(po[:, j, :], attnT[:, 2 * j + b, :],
                                         v16t[:, it + b, :],
                                         start=(j == 0 and b == 0),
                                         stop=(j == 1 and b == 1),
                                         skip_group_check=True)
                rt = rts[it]
                h0 = 2 * hp
                nc.vector.reciprocal(rt[:, h0:h0 + 2], po[:, :, 64:65])
                o16 = o16p.tile([128, 2, 64], F16, tag="o16")
                nc.vector.tensor_scalar_mul(o16[:, 0, :], po[:, 0, 0:64],
                                            rt[:, h0:h0 + 1])
                nc.scalar.activation(o16[:, 1, :], po[:, 1, 0:64],
                                     AF.Copy, scale=rt[:, h0 + 1:h0 + 2])
                if debug and it == 0:
                    nc.vector.tensor_copy(dbg_o16[:, h0:h0 + 2, :], o16[:])
                # oT chunk hp = this pair's o16 transposed
                pt = ps_t.tile([128, 4, 128], F16, tag="tp")
                nc.tensor.transpose(pt[:, 0, :],
                                    o16[:].rearrange("p a b -> p (a b)"),
                                    id16[:])
                oTc = o16p.tile([128, 128], F16, tag="oTc")
                nc.scalar.activation(oTc[:], pt[:, 0, :], AF.Copy)
                # streamed outproj: accumulate this chunk into all 8 nt
                # accumulators (packed 4-per-bank; only the very first matmul
                # into each bank uses start=True)
                pfs = st[("pf", it)]
                for nt in range(8):
                    nc.tensor.matmul(
                        pfs[nt // 4][:, nt % 4, :],
                        w2[:, hp, 128 * nt:128 * (nt + 1)], oTc[:],
                        start=(hp == 0 and nt % 4 == 0), stop=(hp == 7),
                        skip_group_check=True)

            def it_epilogue(it):
                pfs = st.pop(("pf", it))
                if debug and it == 0:
                    nc.gpsimd.dma_start(dbg["dbg_o16"], dbg_o16[:])
                for nt in range(8):
                    pf = pfs[nt // 4][:, nt % 4, :]
                    ot = outp.tile([128, 128], F16, tag="ot")
                    nc.scalar.activation(ot[:], pf, AF.Identity,
                                         bias=b2[:, nt:nt + 1], scale=1.0)
                    nc.sync.dma_start(out_d[nt, it], ot[:])

            # ---- pipelined driver: A one pair ahead of B, ahead of C ----
            for s in range(18):
                if s < 16:
                    if s % 8 == 0:
                        st[("pf", s // 8)] = [
                            ps_f.tile([128, 4, 128], F32, tag=f"pf{g}",
                                      name=f"pf{s // 8}{g}") for g in range(2)]
                    stage_a(2 * s)
                    stage_a(2 * s + 1)
                if 1 <= s <= 16:
                    stage_b(s - 1)
                if s >= 2:
                    stage_c(s - 2)
                    if s - 2 == 7:
                        it_epilogue(0)
                    elif s - 2 == 15:
                        it_epilogue(1)
            _stk.close()

    nc.compile()
    return nc


def _prep_inputs(x, Wqkv, bqkv, Wout, bout):
    x = np.asarray(x, dtype=np.float32)
    Wqkv = np.asarray(Wqkv, dtype=np.float32)
    bqkv = np.asarray(bqkv, dtype=np.float32)
    Wout = np.asarray(Wout, dtype=np.float32)
    bout = np.asarray(bout, dtype=np.float32)

    sq = np.sqrt(np.float32(D))
    W1 = Wqkv.copy()
    b1 = bqkv.copy()
    W1[2 * D:] *= sq
    b1[2 * D:] *= sq
    # [p, t, c, o] = W1[128t+o, 128c+p]
    w1T = np.ascontiguousarray(
        W1.reshape(9, 128, 8, 128).transpose(3, 0, 2, 1).reshape(128, 9 * 8 * 128)
    ).astype(np.float16)
    b1t = np.ascontiguousarray(b1.reshape(9, 128).T)          # [128, 9]
    w2T = np.ascontiguousarray(
        Wout.T.reshape(8, 128, 1024).transpose(1, 0, 2).reshape(128, 8 * 1024)
    ).astype(np.float16)
    b2t = np.ascontiguousarray(bout.reshape(8, 128).T)        # [128, 8]

    pi = np.arange(128)[:, None]
    fj = np.arange(256)[None, :]
    std = np.where((fj > pi) & (fj <= pi + 128), 0.0, -60000.0).astype(np.float16)
    edge = np.where((fj > pi) & (fj <= pi + 128) & (fj >= 128), 0.0,
                    -60000.0).astype(np.float16)
    ident = np.eye(128, dtype=np.float16)

    in_maps = []
    for c in range(NC):
        r0 = c * RPC
        xs = np.zeros((XW, DM), np.float32)
        lo = max(0, r0 - HALO)
        xs[HALO - (r0 - lo):HALO + RPC] = x[lo:r0 + RPC]
        xTc = np.ascontiguousarray(
            xs.T.reshape(8, 128, XW).transpose(1, 0, 2).reshape(128, 8 * XW)
        ).astype(np.float16)
        m0 = edge if c == 0 else std
        mc = np.ascontiguousarray(
            np.stack([np.concatenate([m0, m0], 1),
                      np.concatenate([std, std], 1)], axis=1))  # [128, 2, 512]
        in_maps.append({
            "xT": xTc, "w1T": w1T, "b1": b1t, "w2T": w2T, "b2": b2t,
            "mask": mc, "ident16": ident,
        })
    return in_maps


def kernel(x, Wqkv, bqkv, Wout, bout):
    if "nc" not in _CACHED:
        _CACHED["nc"] = _build()
    nc = _CACHED["nc"]
    in_maps = _prep_inputs(x, Wqkv, bqkv, Wout, bout)
    res = run_bass_kernel_spmd(nc, in_maps, list(range(NC)))
    out = np.empty((S, DM), np.float32)
    for c in range(NC):
        outT = res.results[c]["outT"]          # [8, 2, 128, 128]
        full = outT.transpose(0, 2, 1, 3).reshape(DM, RPC)
        out[c * RPC:(c + 1) * RPC] = full.T.astype(np.float32)
    return out


if __name__ == "__main__":
    rng = np.random.default_rng(0)
    ins = {
        "x": rng.standard_normal((S, DM)).astype(np.float32),
        "Wqkv": (rng.standard_normal((1152, DM)) / 32).astype(np.float32),
        "bqkv": (rng.standard_normal((1152,)) * 0.01).astype(np.float32),
        "Wout": (rng.standard_normal((DM, DM)) / 32).astype(np.float32),
        "bout": (rng.standard_normal((DM,)) * 0.01).astype(np.float32),
    }
    out = kernel(**ins)
    print("kernel ran, out shape", out.shape)
